# revision 23
# baseline (speedup 1.0000x reference)
"""Trainium2 Bass kernel for the spiking conv encoder (nn_Encoder_15410342658418).

Shapes (hardcoded): spike [8,2,128,128,32] -> out [8,32,64,64,32].
Data-parallel over batch N=8, one sample per NeuronCore.

Structure (v5, t-outer, DVE main chain + Pool side chain):
  * HOST pre-applies the exponential current filter along t
    (cur[t] = 0.75 cur[t-1] + x[t]; linear, commutes with the conv), so the
    device matmul directly produces cur[t] and no on-device scan is needed.
  * im2col is materialized on host t-outermost: x[72, T, 2048] per core;
    cols [0:1024] / [1024:2048] are the f16 hi/lo pair of the 1024 pixels
    (yg,x). Weights are split the same way; conv = Whi*xhi + Whi*xlo +
    Wlo*xhi accumulated in f32 PSUM (~22-bit effective mantissa; anything
    narrower flips spikes past the 2e-2 gate - f16/bf16/tf32-class all fail).
  * per t, 3 matmul trios: cols [WA:1024] -> PSUM tile zpB (feeds the Pool
    chain via an ACT staging copy; Pool cannot read PSUM), cols [0:WA] ->
    zpA (read directly by the DVE chain). Separate PSUM pools decouple the
    two chains' semaphore ordering.
  * pixels [0:WA], DVE (STT is a DVE-only instruction):
      u = 0.9*V + cur (STT, in1=PSUM);  V = (u<1)*u (STT)
      sig = Sign(1-u) -> fp8 on ACT; spike <=> sig <= 0.
  * pixels [WA:1024], 3-op Pool chain:
      uB = stageB[t-1] + curB      (TT add)
      m9 = (uB < 1) * 0.9          (TS; f32 keeps the 0.9 decay exact)
      stageB[t] = uB * m9          (TT mult; the 0.9*V state, also the
                                    output: == +-0.0 exactly iff spike)
  * in-DMA per t (t=0 split hi/lo for faster start), out-DMAs in tapered
    half-chunks; weight DMA rides the ACT queue so the SP queue starts on
    input immediately.
  * HOST decodes spikes and applies the per-channel delay interpolation
    out[t] = (1-f) s[t] + f s[t-1] (delay in [0,1)).
"""

import numpy as np
from scipy.signal import lfilter

import concourse.bacc as bacc
import concourse.bass_utils as bass_utils
import concourse.tile as tile
from concourse import mybir

N, C, H, W, T = 8, 2, 128, 128, 32
CH = 32
Hp, Wp = 64, 64
CUR_DECAY = 0.25
VOLT_DECAY = 0.1
YB = 4                  # output rows per partition block
NYG = Hp // YB          # 16 y-groups
K = 72                  # contraction rows: (kx, c, ky, yb)
Q = NYG * Wp            # 1024 pixels per t-slice
WB = 276                # Pool side-chain width (pixels [Q-WB:Q])
WA = Q - WB             # DVE main-chain width
OUT_SIZES = (8, 8, 8, 4, 2, 1, 1)   # tapered output chunks (sum = T)

_COMPILED = None


def _build_program():
    nc = bacc.Bacc("TRN2", target_bir_lowering=False, debug=False, num_devices=N)
    f16 = mybir.dt.float16
    f8 = mybir.dt.float8e4
    f32 = mybir.dt.float32
    x_d = nc.dram_tensor("x", [K, T, 2 * Q], f16, kind="ExternalInput")
    wblk_d = nc.dram_tensor("wblk", [K, 2, 128], f16, kind="ExternalInput")
    out_d = nc.dram_tensor("out", [128, T, WA], f8, kind="ExternalOutput")
    outb_d = nc.dram_tensor("outb", [128, T, WB], f32, kind="ExternalOutput")

    from contextlib import ExitStack

    with tile.TileContext(nc) as tc, ExitStack() as ctx:
        _kernel_body(ctx, tc, x_d.ap(), wblk_d.ap(), out_d.ap(), outb_d.ap())
    nc.compile()
    return nc


def _kernel_body(ctx, tc, x, wblk, out, outb):
    nc = tc.nc
    f32 = mybir.dt.float32
    f16 = mybir.dt.float16
    f8 = mybir.dt.float8e4
    Alu = mybir.AluOpType
    Act = mybir.ActivationFunctionType

    consts = ctx.enter_context(tc.tile_pool(name="consts", bufs=1))
    rhsp = ctx.enter_context(tc.tile_pool(name="rhsp", bufs=8))
    psa = ctx.enter_context(tc.tile_pool(name="psa", bufs=3, space="PSUM"))
    psb = ctx.enter_context(tc.tile_pool(name="psb", bufs=2, space="PSUM"))
    upool = ctx.enter_context(tc.tile_pool(name="upool", bufs=3))
    sigp = ctx.enter_context(tc.tile_pool(name="sigp", bufs=3))
    stagep = ctx.enter_context(tc.tile_pool(name="stagep", bufs=3))
    bpool = ctx.enter_context(tc.tile_pool(name="bpool", bufs=2))

    wblk_t = consts.tile([K, 2, 128], f16)
    # keep the SP queue free for the first input chunk
    nc.scalar.dma_start(out=wblk_t, in_=wblk)
    whi, wlo = wblk_t[:, 0], wblk_t[:, 1]

    V = consts.tile([128, WA], f32, name="V")
    nc.vector.memset(V, 0.0)
    bzero = consts.tile([128, WB], f32, name="bzero")
    nc.gpsimd.memset(bzero, 0.0)
    prevB = bzero

    out_starts = {}
    t0 = 0
    for s in OUT_SIZES:
        out_starts[t0] = s
        t0 += s
    assert t0 == T

    sig = stage = None
    obase = osz = 0
    pend = None

    def _emit_sig(u_t, sig_t, oi_t, ob, os_):
        nc.scalar.activation(
            out=sig_t[:, oi_t], in_=u_t, func=Act.Sign, bias=1.0, scale=-1.0
        )
        if os_ >= 4:
            h = os_ // 2
            if oi_t == h - 1:
                nc.scalar.dma_start(out=out[:, ob : ob + h, :], in_=sig_t[:, 0:h])
            elif oi_t == os_ - 1:
                nc.scalar.dma_start(
                    out=out[:, ob + h : ob + os_, :], in_=sig_t[:, h:os_]
                )
        elif oi_t == os_ - 1:
            nc.scalar.dma_start(out=out[:, ob : ob + os_, :], in_=sig_t[:, 0:os_])

    for tt in range(T):
        rhs = rhsp.tile([K, 2 * Q], f16, tag="rhs", name=f"rhs{tt}")
        if tt == 0:
            # four pieces, B-region cols first, so the first zpB trio (and
            # with it the whole Pool chain) starts as early as possible
            nc.sync.dma_start(out=rhs[:, WA:Q], in_=x[:, 0, WA:Q])
            nc.sync.dma_start(out=rhs[:, Q + WA : 2 * Q], in_=x[:, 0, Q + WA : 2 * Q])
            # A-region pieces ride the Pool SWDGE path, parallel to HWDGE,
            # while the Pool engine is still idle
            nc.gpsimd.dma_start(out=rhs[:, 0:WA], in_=x[:, 0, 0:WA])
            nc.gpsimd.dma_start(out=rhs[:, Q : Q + WA], in_=x[:, 0, Q : Q + WA])
        elif tt == 1:
            nc.gpsimd.dma_start(out=rhs, in_=x[:, tt, :])
        else:
            nc.sync.dma_start(out=rhs, in_=x[:, tt, :])
        if tt in out_starts:
            osz = out_starts[tt]
            obase = tt
            sig = sigp.tile([128, 8, WA], f8, tag="sig", name=f"sig{tt}")
            stage = stagep.tile([128, 8, WB], f32, tag="stage", name=f"stage{tt}")
        oi = tt - obase

        zpA = psa.tile([128, WA], f32, tag="zpA", name=f"zpA{tt}")
        zpB = psb.tile([128, WB], f32, tag="zpB", name=f"zpB{tt}")
        # B trio first so the Pool chain's staging copy can start earliest
        for lo, hi, dst in (
            (WA, Q, zpB),
            (0, 512, zpA[:, 0:512]),
            (512, WA, zpA[:, 512:WA]),
        ):
            xhi = rhs[:, lo:hi]
            xlo = rhs[:, Q + lo : Q + hi]
            nc.tensor.matmul(dst, lhsT=whi, rhs=xhi, start=True, stop=False)
            nc.tensor.matmul(dst, lhsT=whi, rhs=xlo, start=False, stop=False)
            nc.tensor.matmul(dst, lhsT=wlo, rhs=xhi, start=False, stop=True)

        curB = bpool.tile([128, WB], f32, tag="curB", name=f"curB{tt}")
        nc.scalar.activation(out=curB, in_=zpB, func=Act.Copy, scale=1.0)

        # ---- A chain (DVE) on [0:WA]
        u = upool.tile([128, WA], f32, tag="u", name=f"u{tt}")
        nc.vector.scalar_tensor_tensor(
            out=u, in0=V, scalar=1.0 - VOLT_DECAY,
            in1=zpA, op0=Alu.mult, op1=Alu.add,
        )
        if tt < T - 1:  # V[31] has no consumer
            nc.vector.scalar_tensor_tensor(
                out=V, in0=u, scalar=1.0, in1=u, op0=Alu.is_lt, op1=Alu.mult,
            )
        # sig for step t-1 is emitted here (one step deferred) so ACT's
        # in-order queue never holds the next curB behind a sig that is
        # still waiting on DVE
        if tt == T - 1:
            # final sig on DVE itself ((u<1) -> {1,0} fp8, same host decode):
            # no cross-engine hop before the last output DMA
            if pend is not None:
                _emit_sig(*pend)
            pend = None
            nc.vector.tensor_scalar(out=sig[:, oi], in0=u, scalar1=1.0,
                                    scalar2=None, op0=Alu.is_lt)
            nc.scalar.dma_start(out=out[:, obase : obase + osz, :], in_=sig[:, 0:osz])
        else:
            if pend is not None:
                _emit_sig(*pend)
            pend = (u, sig, oi, obase, osz)

        # ---- B chain (Pool) on [WA:Q]
        uB = bpool.tile([128, WB], f32, tag="uB", name=f"uB{tt}")
        nc.gpsimd.tensor_tensor(out=uB, in0=prevB, in1=curB, op=Alu.add)
        m9 = bpool.tile([128, WB], f32, tag="m9", name=f"m9{tt}")
        nc.gpsimd.tensor_scalar(
            out=m9, in0=uB, scalar1=1.0, scalar2=1.0 - VOLT_DECAY,
            op0=Alu.is_lt, op1=Alu.mult,
        )
        nc.gpsimd.tensor_tensor(out=stage[:, oi], in0=uB, in1=m9, op=Alu.mult)
        prevB = stage[:, oi]

        if osz >= 4:
            h = osz // 2
            if oi == h - 1:
                nc.scalar.dma_start(out=outb[:, obase : obase + h, :], in_=stage[:, 0:h])
            elif oi == osz - 1:
                nc.scalar.dma_start(
                    out=outb[:, obase + h : obase + osz, :], in_=stage[:, h:osz]
                )
        elif oi == osz - 1:
            # last small chunks ride the idle SP queue in parallel with the
            # deferred sig DMAs on ACT
            nc.sync.dma_start(out=outb[:, obase : obase + osz, :], in_=stage[:, 0:osz])
    if pend is not None:
        _emit_sig(*pend)


def _host_prep(spike, weight_v, weight_g, delay):
    spike = np.asarray(spike, dtype=np.float32)
    weight_v = np.asarray(weight_v, dtype=np.float32)
    weight_g = np.asarray(weight_g, dtype=np.float32)

    vnorm = np.sqrt((weight_v * weight_v).sum(axis=(1, 2, 3), keepdims=True))
    wn = (weight_g[:, None, None, None] * weight_v / vnorm).astype(np.float32)

    # lhsT [72, 2, 128]: row kx*24 + c*12 + ky*4 + yb -> [hi/lo][yb*32 + ch]
    wblk = np.zeros((K, 128), dtype=np.float32)
    for yb in range(YB):
        for kx in range(3):
            for c in range(C):
                for ky in range(3):
                    row = kx * 24 + c * 12 + ky * 4 + yb
                    wblk[row, yb * 32 : (yb + 1) * 32] = wn[:, c, ky, kx]
    whi = wblk.astype(np.float16)
    wlo = (wblk - whi.astype(np.float32)).astype(np.float16)
    wpair = np.stack([whi, wlo], axis=1)  # [72, 2, 128]

    # host current prefilter along t (linear; commutes with the conv)
    curx = lfilter([1.0], [1.0, -(1.0 - CUR_DECAY)], spike, axis=-1)
    curx = np.ascontiguousarray(curx.astype(np.float32))

    # t-outer im2col: xr[n, p=(kx,c,ky,yb), t, (yg,x)] then f16 hi/lo cols
    xpad = np.pad(curx, ((0, 0), (0, 0), (1, 0), (1, 0), (0, 0)))
    xr = np.empty((N, K, T, Q), dtype=np.float32)
    for kx in range(3):
        for ky in range(3):
            for yb in range(YB):
                rows = 8 * np.arange(NYG) + 2 * yb + ky
                # [n, c, yg, x, t] -> [n, c, t, yg*x]
                blk = xpad[:, :, rows, kx : kx + 2 * Wp : 2, :]
                blk = blk.transpose(0, 1, 4, 2, 3).reshape(N, C, T, Q)
                for c in range(C):
                    xr[:, kx * 24 + c * 12 + ky * 4 + yb] = blk[:, c]
    xhi = xr.astype(np.float16)
    xlo = (xr - xhi.astype(np.float32)).astype(np.float16)
    xpairs = np.concatenate([xhi, xlo], axis=-1)  # [n, 72, T, 2048]
    return xpairs, wpair


def _host_post(outs_a, outs_b, delay):
    delay = np.asarray(delay, dtype=np.float32)
    f = (delay - np.floor(delay)).astype(np.float32)
    fb = f[None, :, None, None, None]
    full = np.empty((N, CH, Hp, Wp, T), dtype=np.float32)
    for n in range(N):
        # region A [128, T, WA] fp8: spike <=> sig <= 0
        ba = np.asarray(outs_a[n]).view(np.uint8)
        sa = (ba == 0) | (ba >= 0x80)
        # region B [128, T, WB] f32: spike <=> stage == +-0.0
        bb = np.asarray(outs_b[n]).view(np.uint32)
        sb = (bb & np.uint32(0x7FFFFFFF)) == 0
        s = np.concatenate([sa, sb], axis=2).astype(np.float32)  # [128, T, Q]
        # [(yb,ch), t, (yg,x)] -> [ch, yg, yb, x, t] -> [ch, y, x, t]
        s = s.reshape(YB, CH, T, NYG, Wp).transpose(1, 3, 0, 4, 2)
        s = s.reshape(CH, Hp, Wp, T)
        s_shift = np.zeros_like(s)
        s_shift[..., 1:] = s[..., :-1]
        full[n] = (1.0 - fb[0]) * s + fb[0] * s_shift
    return full


def kernel(spike, weight_v, weight_g, delay):
    global _COMPILED
    if _COMPILED is None:
        _COMPILED = _build_program()
    nc = _COMPILED

    xpairs, wpair = _host_prep(spike, weight_v, weight_g, delay)
    in_maps = [
        {"x": np.ascontiguousarray(xpairs[n]), "wblk": wpair}
        for n in range(N)
    ]
    res = bass_utils.run_bass_kernel_spmd(nc, in_maps, core_ids=list(range(N)))
    return _host_post(
        [r["out"] for r in res.results],
        [r["outb"] for r in res.results],
        delay,
    )
